# revision 11
# baseline (speedup 1.0000x reference)
"""Multi-head attention (with the repo's k=q bug) on 8 Trainium2 NeuronCores.

Reference computation (B=2, S=2048, D=512, H=8, DK=64):
    q = query @ Wq.T ; v = value @ Wv.T          (k-projection is dead code)
    qh = q.reshape(B, H, S, DK)  (raw view: head h = a contiguous 256-row slab
                                  of q, re-chunked into rows of 64)
    kh = qh                      (repo bug: key = query.view(...))
    scores = qh @ qh^T / 8 ; mask ; softmax ; x = attn @ vh
    out = x.transpose/reshape @ Wo.T

Sharding: core c owns head h=c for both batches (2 (b,h) pairs/core).

v2 layout: everything bf16 on the matmul path, both batches packed into the
two partition halves (b0 -> partitions 0:64, b1 -> 64:128) so score matmuls
run as concurrent PE quadrant pairs and each exp instruction covers both
batches.  Per (j-tile t, i-chunk n) the device computes, score-transposed:
    E_T = exp(S_T/8 - 20) * tri-mask   (only the diagonal 128-col block of a
                                        diagonal tile needs masking)
    [x_unnorm^T; l] += [vh | ones]^T augmented PV matmul   (per batch)
Chunk epilogue: po = x_unnorm^T.T @ Wo_h.T written bf16; host divides by l
and sums partials over heads/cores.  Fully-masked j-tiles are skipped and
diagonal tiles are column-restricted (causal structure verified on host;
non-causal masks fall back to numpy).
"""

import math
import sys

import numpy as np

sys.path.insert(0, "/opt/trn_rl_repo")

B, S, D, H, DK = 2, 2048, 512, 8, 64
NCORES = 8
SLAB = S // H          # 256 query rows per head-slab
CHUNK = 512            # i-chunk width
JT = 128               # j-tile height
NCHUNK = S // CHUNK    # 4
NJT = S // JT          # 16
KT = D // 128          # 4 k-tiles over the projections' contraction dim
EXP_BIAS = -20.0


def _enable_ldw_opt():
    """Flip walrus --enable-ldw-opt: elides back-to-back LDWEIGHTS of the
    same stationary operand."""
    from concourse import bass_utils
    if getattr(bass_utils, "_ldw_patched", False):
        return
    orig = bass_utils.run_command

    def run_command(argv, **kw):
        import subprocess
        try:
            return orig(argv, **kw)
        except subprocess.CalledProcessError as e:
            err = e.stderr if isinstance(e.stderr, str) else (
                e.stderr.decode() if e.stderr else "")
            out = e.stdout if isinstance(e.stdout, str) else (
                e.stdout.decode() if e.stdout else "")
            sys.stderr.write("WALRUS FAIL STDERR:\n" + err[-4000:] + "\n")
            sys.stderr.write("WALRUS FAIL STDOUT:\n" + out[-4000:] + "\n")
            raise

    bass_utils.run_command = run_command
    bass_utils._ldw_patched = True

_cache: dict = {}


def _build_causal():
    import concourse.bass as bass
    import concourse.tile as tile
    from concourse import bacc, mybir

    _enable_ldw_opt()

    f32 = mybir.dt.float32
    bf16 = mybir.dt.bfloat16
    nc = bacc.Bacc("TRN2", target_bir_lowering=False, debug=False,
                   num_devices=NCORES)

    # inputs (all bf16; batches packed side by side in the free dim)
    qT = nc.dram_tensor("qT", [D, 2 * SLAB], bf16, kind="ExternalInput").ap()
    vT = nc.dram_tensor("vT", [D, 2 * SLAB], bf16, kind="ExternalInput").ap()
    wqT = nc.dram_tensor("wqT", [D, D], bf16, kind="ExternalInput").ap()
    wvT = nc.dram_tensor("wvT", [D, D], bf16, kind="ExternalInput").ap()
    woT = nc.dram_tensor("woT", [DK, D], bf16, kind="ExternalInput").ap()
    mtri = nc.dram_tensor("mtri", [JT, JT], bf16, kind="ExternalInput").ap()
    po = nc.dram_tensor("po", [B, S, D], bf16, kind="ExternalOutput").ap()
    lo = nc.dram_tensor("lo", [B, 1, S], f32, kind="ExternalOutput").ap()
    # DRAM staging for the SBUF repartition gathers (qhT / vh layouts)
    qstage = nc.dram_tensor("qstage", [4, 128, 2 * SLAB], bf16,
                            kind="Internal").ap()
    vstage = nc.dram_tensor("vstage", [4, 128, D], bf16, kind="Internal").ap()

    with tile.TileContext(nc) as tc:
        with (
            tc.tile_pool(name="const", bufs=1) as constp,
            tc.tile_pool(name="acts", bufs=1) as actp,
            tc.tile_pool(name="qhT", bufs=1) as qhTp,
            tc.tile_pool(name="vh", bufs=1) as vhp,
            tc.tile_pool(name="qc", bufs=2) as qcp,
            tc.tile_pool(name="eT", bufs=3) as eTp,
            tc.tile_pool(name="xT", bufs=2) as xTp,
            tc.tile_pool(name="fo", bufs=4) as fop,
            tc.tile_pool(name="psS", bufs=2, space="PSUM") as psS,
            tc.tile_pool(name="psX", bufs=1, space="PSUM") as psX,
            tc.tile_pool(name="psM", bufs=1, space="PSUM") as psM,
        ):
            # ---- input DMA (bf16), spread across engine rings -----------
            wq_sb, wv_sb = [], []
            qT_sb, vT_sb = [], []
            for k in range(KT):
                t = constp.tile([128, D], bf16, tag=f"wq{k}")
                nc.sync.dma_start(t[:], wqT[128 * k:128 * (k + 1), :])
                wq_sb.append(t)
                t = actp.tile([128, 2 * SLAB], bf16, tag=f"qt{k}")
                nc.scalar.dma_start(t[:], qT[128 * k:128 * (k + 1), :])
                qT_sb.append(t)
            for k in range(KT):
                t = constp.tile([128, D], bf16, tag=f"wv{k}")
                nc.gpsimd.dma_start(t[:], wvT[128 * k:128 * (k + 1), :])
                wv_sb.append(t)
                t = actp.tile([128, 2 * SLAB], bf16, tag=f"vt{k}")
                nc.scalar.dma_start(t[:], vT[128 * k:128 * (k + 1), :])
                vT_sb.append(t)
            # Wo^T head slice duplicated into both partition halves
            wo_sb = constp.tile([128, D], bf16, tag="wo")
            nc.gpsimd.dma_start(wo_sb[0:64, :], woT[:, :])
            nc.gpsimd.dma_start(wo_sb[64:128, :], woT[:, :])
            # triangular mask pattern (p <= f) for diagonal 128x128 blocks
            mt_sb = constp.tile([JT, JT], bf16, tag="mtri")
            nc.gpsimd.dma_start(mt_sb[:], mtri[:, :])
            exp_bias = constp.tile([128, 1], f32, tag="ebias")
            nc.gpsimd.memset(exp_bias[:], EXP_BIAS)

            # ---- q projection -> qstage -> qhT gather -------------------
            # qhT [128, S]: partitions 0:64 = b0 dk, 64:128 = b1 dk;
            # col j = 8*row + cidx  (raw-view head split)
            qhT = qhTp.tile([128, S], bf16, tag="qhT")
            qhT_v = qhT.rearrange("p (r c) -> p r c", c=H)
            for jg in range(4):
                psq = psM.tile([128, 2 * SLAB], f32, tag=f"psf{jg % 2}")
                for k in range(KT):
                    nc.tensor.matmul(
                        psq[:], wq_sb[k][:, 128 * jg:128 * (jg + 1)],
                        qT_sb[k][:], start=(k == 0), stop=(k == KT - 1))
                qc = qcp.tile([128, 2 * SLAB], bf16, tag="qc")
                nc.vector.tensor_copy(qc[:], psq[:])
                nc.sync.dma_start(qstage[jg, :, :], qc[:])
                # gather: dst [p=64b+d2, r] <- qstage[jg][64jl+d2, 256b+r]
                src = qstage[jg].rearrange(
                    "(jl dd) (b r) -> jl b dd r", jl=2, b=2)
                for jl in range(2):
                    for b in range(2):
                        nc.sync.dma_start(
                            qhT_v[64 * b:64 * (b + 1), :, 2 * jg + jl],
                            src[jl, b])

            # ---- v projection -> vstage -> vh gather --------------------
            # vh_all[b] [128, 16*65]: tile t at cols 65t:65t+64, ones at 65t+64
            vh_all = []
            for b in range(B):
                t = vhp.tile([128, NJT * (DK + 1)], bf16, tag=f"vha{b}")
                vv = t.rearrange("p (t c) -> p t c", c=DK + 1)
                nc.gpsimd.memset(vv[:, :, DK:DK + 1], 1.0)
                vh_all.append(t)
            vh_v = [t.rearrange("p (t c) -> p t c", c=DK + 1) for t in vh_all]
            for rh in range(4):           # rh = 2*b + half
                psv = psM.tile([128, D], f32, tag=f"psf{rh % 2}")
                b, half = rh // 2, rh % 2
                for k in range(KT):
                    nc.tensor.matmul(
                        psv[:],
                        vT_sb[k][:, 256 * b + 128 * half:
                                 256 * b + 128 * (half + 1)],
                        wv_sb[k][:], start=(k == 0), stop=(k == KT - 1))
                vc = qcp.tile([128, D], bf16, tag="vc")
                nc.vector.tensor_copy(vc[:], psv[:])
                nc.sync.dma_start(vstage[rh, :, :], vc[:])
                # gather 8 vh tiles: dst[p=(8*rm+c8), d]
                #   <- vstage[rh][16*tl+rm, 64*c8+d]
                # (dst flat order (rm, c8, d) matches src (rm, (c8 d)))
                tb = 8 * half
                src = vstage[rh].rearrange("(tl rm) f -> tl rm f", tl=8)
                for tl in range(8):
                    nc.sync.dma_start(vh_v[b][:, tb + tl, 0:DK], src[tl])

            # ---- attention over i-chunks --------------------------------
            for n in range(NCHUNK):
                n_t = 4 * n + 4
                psx0 = psX.tile([DK + 1, CHUNK], f32, tag="psx0")
                psx1 = psX.tile([DK + 1, CHUNK], f32, tag="psx1")
                for t_ in range(n_t):
                    s_ = t_ - 4 * n
                    off = max(0, s_) * JT
                    pss = psS.tile([128, 2 * CHUNK], f32, tag="pss")
                    nc.tensor.matmul(
                        pss[:, off:CHUNK],
                        qhT[0:64, JT * t_:JT * (t_ + 1)],
                        qhT[0:64, CHUNK * n + off:CHUNK * (n + 1)],
                        start=True, stop=True, tile_position=(0, 0))
                    nc.tensor.matmul(
                        pss[:, CHUNK + off:2 * CHUNK],
                        qhT[64:128, JT * t_:JT * (t_ + 1)],
                        qhT[64:128, CHUNK * n + off:CHUNK * (n + 1)],
                        start=True, stop=True, tile_position=(64, 0))
                    eT = eTp.tile([128, 2 * CHUNK], bf16, tag="eT")
                    nc.scalar.activation(
                        eT[:, off:], pss[:, off:],
                        mybir.ActivationFunctionType.Exp,
                        bias=exp_bias[:], scale=1.0 / math.sqrt(DK))
                    if s_ >= 0:
                        # only the diagonal 128-col block needs masking
                        sl0 = eT[:, off:off + JT]
                        nc.vector.tensor_mul(sl0, sl0, mt_sb[:])
                        sl1 = eT[:, CHUNK + off:CHUNK + off + JT]
                        nc.gpsimd.tensor_mul(sl1, sl1, mt_sb[:])
                    nc.tensor.matmul(
                        psx0[:, off:], vh_all[0][:, 65 * t_:65 * t_ + 65],
                        eT[:, off:CHUNK],
                        start=(t_ == 0), stop=(t_ == n_t - 1),
                        skip_group_check=True)
                    nc.tensor.matmul(
                        psx1[:, off:], vh_all[1][:, 65 * t_:65 * t_ + 65],
                        eT[:, CHUNK + off:2 * CHUNK],
                        start=(t_ == 0), stop=(t_ == n_t - 1),
                        skip_group_check=True)

                # ---- chunk epilogue: evacuate x^T + l, project, store ---
                xT = xTp.tile([128, CHUNK], bf16, tag="xT")
                nc.vector.tensor_copy(xT[0:64, :], psx0[0:64, :])
                nc.vector.tensor_copy(xT[64:128, :], psx1[0:64, :])
                ls0 = xTp.tile([1, CHUNK], f32, tag="ls0")
                ls1 = xTp.tile([1, CHUNK], f32, tag="ls1")
                nc.vector.tensor_copy(ls0[:], psx0[64:65, :])
                nc.vector.tensor_copy(ls1[:], psx1[64:65, :])
                nc.sync.dma_start(lo[0, :, CHUNK * n:CHUNK * (n + 1)],
                                  ls0[:])
                nc.sync.dma_start(lo[1, :, CHUNK * n:CHUNK * (n + 1)],
                                  ls1[:])
                for u in range(CHUNK // 128):
                    psf0 = psM.tile([128, D], f32, tag="psf0")
                    psf1 = psM.tile([128, D], f32, tag="psf1")
                    nc.tensor.matmul(
                        psf0[:], xT[0:64, 128 * u:128 * (u + 1)],
                        wo_sb[0:64, :], start=True, stop=True,
                        tile_position=(0, 0))
                    nc.tensor.matmul(
                        psf1[:], xT[64:128, 128 * u:128 * (u + 1)],
                        wo_sb[64:128, :], start=True, stop=True,
                        tile_position=(64, 0))
                    fo0 = fop.tile([128, D], bf16, tag="fo0")
                    fo1 = fop.tile([128, D], bf16, tag="fo1")
                    nc.vector.tensor_copy(fo0[:], psf0[:])
                    nc.vector.tensor_copy(fo1[:], psf1[:])
                    r0 = CHUNK * n + 128 * u
                    nc.sync.dma_start(po[0, r0:r0 + 128, :], fo0[:])
                    nc.sync.dma_start(po[1, r0:r0 + 128, :], fo1[:])
    nc.compile()
    return nc


def _tri_pattern():
    import ml_dtypes
    p = np.arange(JT)[:, None]
    f = np.arange(JT)[None, :]
    return (p <= f).astype(ml_dtypes.bfloat16)


def _numpy_fallback(query, key, value, mask, Wq, Wk, Wv, Wo):
    q = query @ Wq.T
    v = value @ Wv.T
    qh = q.reshape(B, H, S, DK)
    vh = v.reshape(B, H, S, DK)
    scores = np.einsum("bhqd,bhkd->bhqk", qh, qh) / math.sqrt(DK)
    scores = np.where(mask == 0, np.float32(-1e9), scores)
    scores = scores - scores.max(axis=-1, keepdims=True)
    e = np.exp(scores)
    attn = e / e.sum(axis=-1, keepdims=True)
    x = np.einsum("bhqk,bhkd->bhqd", attn, vh)
    x = x.transpose(0, 2, 1, 3).reshape(B, S, H * DK)
    return (x @ Wo.T).astype(np.float32)


def _run_device(query, value, Wq, Wv, Wo, trace=False):
    import ml_dtypes
    from concourse.bass_utils import run_bass_kernel_spmd

    if "nc" not in _cache:
        _cache["nc"] = _build_causal()
    nc = _cache["nc"]

    bf = ml_dtypes.bfloat16
    mtri = _tri_pattern()
    wqT = np.ascontiguousarray(Wq.T).astype(bf)
    wvT = np.ascontiguousarray(Wv.T).astype(bf)
    in_maps = []
    for c in range(NCORES):
        r0 = SLAB * c
        qs = query[:, r0:r0 + SLAB, :]      # [B, SLAB, D]
        vs = value[:, r0:r0 + SLAB, :]
        in_maps.append({
            # [D, 2*SLAB]: b0 cols then b1 cols
            "qT": np.ascontiguousarray(
                qs.transpose(2, 0, 1).reshape(D, 2 * SLAB)).astype(bf),
            "vT": np.ascontiguousarray(
                vs.transpose(2, 0, 1).reshape(D, 2 * SLAB)).astype(bf),
            "wqT": wqT,
            "wvT": wvT,
            "woT": np.ascontiguousarray(
                Wo[:, DK * c:DK * (c + 1)].T).astype(bf),
            "mtri": mtri,
        })
    res = run_bass_kernel_spmd(nc, in_maps, core_ids=list(range(NCORES)),
                               trace=trace)
    out = np.zeros((B, S, D), dtype=np.float32)
    for c in range(NCORES):
        pc = res.results[c]
        out += pc["po"].astype(np.float32) / \
            pc["lo"].reshape(B, S, 1)
    return out, res


_TRIL = None


def kernel(query, key, value, mask, Wq, Wk, Wv, Wo):
    global _TRIL
    query = np.asarray(query, dtype=np.float32)
    value = np.asarray(value, dtype=np.float32)
    mask = np.asarray(mask)
    Wq = np.asarray(Wq, dtype=np.float32)
    Wv = np.asarray(Wv, dtype=np.float32)
    Wo = np.asarray(Wo, dtype=np.float32)

    if _TRIL is None:
        _TRIL = np.tril(np.ones((S, S), dtype=np.int64))
    m2 = mask.reshape(S, S)
    if not np.array_equal(m2 != 0, _TRIL != 0):
        return _numpy_fallback(query, np.asarray(key), value, mask,
                               Wq, np.asarray(Wk), Wv, Wo)

    out, _ = _run_device(query, value, Wq, Wv, Wo)
    return out


# revision 12
# speedup vs baseline: 3.6128x; 3.6128x over previous
"""Multi-head attention (with the repo's k=q bug) on 8 Trainium2 NeuronCores.

Reference computation (B=2, S=2048, D=512, H=8, DK=64):
    q = query @ Wq.T ; v = value @ Wv.T          (k-projection is dead code)
    qh = q.reshape(B, H, S, DK)  (raw view: head h = a contiguous 256-row slab
                                  of q, re-chunked into rows of 64)
    kh = qh                      (repo bug: key = query.view(...))
    scores = qh @ qh^T / 8 ; mask ; softmax ; x = attn @ vh
    out = x.transpose/reshape @ Wo.T

Sharding: core c owns head h=c for both batches (2 (b,h) pairs/core).

v2 layout: everything bf16 on the matmul path, both batches packed into the
two partition halves (b0 -> partitions 0:64, b1 -> 64:128) so score matmuls
run as concurrent PE quadrant pairs and each exp instruction covers both
batches.  Per (j-tile t, i-chunk n) the device computes, score-transposed:
    E_T = exp(S_T/8 - 20) * tri-mask   (only the diagonal 128-col block of a
                                        diagonal tile needs masking)
    [x_unnorm^T; l] += [vh | ones]^T augmented PV matmul   (per batch)
Chunk epilogue: po = x_unnorm^T.T @ Wo_h.T written bf16; host divides by l
and sums partials over heads/cores.  Fully-masked j-tiles are skipped and
diagonal tiles are column-restricted (causal structure verified on host;
non-causal masks fall back to numpy).
"""

import math
import sys

import numpy as np

sys.path.insert(0, "/opt/trn_rl_repo")

B, S, D, H, DK = 2, 2048, 512, 8, 64
NCORES = 8
SLAB = S // H          # 256 query rows per head-slab
CHUNK = 512            # i-chunk width
JT = 128               # j-tile height
NCHUNK = S // CHUNK    # 4
NJT = S // JT          # 16
KT = D // 128          # 4 k-tiles over the projections' contraction dim
EXP_BIAS = -20.0


def _enable_ldw_opt():
    """Flip walrus --enable-ldw-opt: elides back-to-back LDWEIGHTS of the
    same stationary operand."""
    from concourse import bass_utils
    if getattr(bass_utils, "_ldw_patched", False):
        return
    orig = bass_utils.run_command

    def run_command(argv, **kw):
        import subprocess
        try:
            return orig(argv, **kw)
        except subprocess.CalledProcessError as e:
            err = e.stderr if isinstance(e.stderr, str) else (
                e.stderr.decode() if e.stderr else "")
            out = e.stdout if isinstance(e.stdout, str) else (
                e.stdout.decode() if e.stdout else "")
            sys.stderr.write("WALRUS FAIL STDERR:\n" + err[-4000:] + "\n")
            sys.stderr.write("WALRUS FAIL STDOUT:\n" + out[-4000:] + "\n")
            raise

    bass_utils.run_command = run_command
    bass_utils._ldw_patched = True

_cache: dict = {}


def _build_causal():
    import concourse.bass as bass
    import concourse.tile as tile
    from concourse import bacc, mybir

    _enable_ldw_opt()

    f32 = mybir.dt.float32
    bf16 = mybir.dt.bfloat16
    nc = bacc.Bacc("TRN2", target_bir_lowering=False, debug=False,
                   num_devices=NCORES)

    # inputs (all bf16; batches packed side by side in the free dim)
    qT = nc.dram_tensor("qT", [D, 2 * SLAB], bf16, kind="ExternalInput").ap()
    vT = nc.dram_tensor("vT", [D, 2 * SLAB], bf16, kind="ExternalInput").ap()
    wqT = nc.dram_tensor("wqT", [D, D], bf16, kind="ExternalInput").ap()
    wvT = nc.dram_tensor("wvT", [D, D], bf16, kind="ExternalInput").ap()
    woT = nc.dram_tensor("woT", [DK, D], bf16, kind="ExternalInput").ap()
    mtri = nc.dram_tensor("mtri", [JT, JT], bf16, kind="ExternalInput").ap()
    po = nc.dram_tensor("po", [B, S, D], bf16, kind="ExternalOutput").ap()
    lo = nc.dram_tensor("lo", [B, 1, S], f32, kind="ExternalOutput").ap()
    # DRAM staging for the SBUF repartition gather (vh layout)
    vstage = nc.dram_tensor("vstage", [4, 128, D], bf16, kind="Internal").ap()

    with tile.TileContext(nc) as tc:
        with (
            tc.tile_pool(name="const", bufs=1) as constp,
            tc.tile_pool(name="acts", bufs=1) as actp,
            tc.tile_pool(name="qhT", bufs=1) as qhTp,
            tc.tile_pool(name="vh", bufs=1) as vhp,
            tc.tile_pool(name="qc", bufs=2) as qcp,
            tc.tile_pool(name="eT", bufs=3) as eTp,
            tc.tile_pool(name="xT", bufs=2) as xTp,
            tc.tile_pool(name="fo", bufs=4) as fop,
            tc.tile_pool(name="psS", bufs=2, space="PSUM") as psS,
            tc.tile_pool(name="psX", bufs=1, space="PSUM") as psX,
            tc.tile_pool(name="psM", bufs=1, space="PSUM") as psM,
        ):
            # ---- input DMA (bf16), spread across engine rings -----------
            wq_sb, wv_sb = [], []
            qT_sb, vT_sb = [], []
            for k in range(KT):
                t = constp.tile([128, D], bf16, tag=f"wq{k}")
                nc.sync.dma_start(t[:], wqT[128 * k:128 * (k + 1), :])
                wq_sb.append(t)
                t = actp.tile([128, 2 * SLAB], bf16, tag=f"qt{k}")
                nc.scalar.dma_start(t[:], qT[128 * k:128 * (k + 1), :])
                qT_sb.append(t)
            for k in range(KT):
                t = constp.tile([128, D], bf16, tag=f"wv{k}")
                nc.gpsimd.dma_start(t[:], wvT[128 * k:128 * (k + 1), :])
                wv_sb.append(t)
                t = actp.tile([128, 2 * SLAB], bf16, tag=f"vt{k}")
                nc.scalar.dma_start(t[:], vT[128 * k:128 * (k + 1), :])
                vT_sb.append(t)
            # Wo^T head slice duplicated into both partition halves
            wo_sb = constp.tile([128, D], bf16, tag="wo")
            nc.gpsimd.dma_start(wo_sb[0:64, :], woT[:, :])
            nc.gpsimd.dma_start(wo_sb[64:128, :], woT[:, :])
            # triangular mask pattern (p <= f) for diagonal 128x128 blocks
            mt_sb = constp.tile([JT, JT], bf16, tag="mtri")
            nc.gpsimd.dma_start(mt_sb[:], mtri[:, :])
            exp_bias = constp.tile([128, 1], f32, tag="ebias")
            nc.gpsimd.memset(exp_bias[:], EXP_BIAS)

            # ---- q projection -> qstage -> qhT gather -------------------
            # qhT [128, S]: partitions 0:64 = b0 dk, 64:128 = b1 dk;
            # col j = 8*row + cidx  (raw-view head split)
            qhT = qhTp.tile([128, S], bf16, tag="qhT")
            qhT_v = qhT.rearrange("p (r c) -> p r c", c=H)
            for jg in range(4):
                psq = psM.tile([128, 2 * SLAB], f32, tag=f"psf{jg % 2}")
                for k in range(KT):
                    nc.tensor.matmul(
                        psq[:], wq_sb[k][:, 128 * jg:128 * (jg + 1)],
                        qT_sb[k][:], start=(k == 0), stop=(k == KT - 1))
                # direct strided evacuation: psq[64jl+d2, 256b+r]
                #   -> qhT_v[64b+d2, r, 2jg+jl]   (16B-stride bf16 writes)
                for jl in range(2):
                    for b in range(2):
                        nc.vector.tensor_copy(
                            qhT_v[64 * b:64 * (b + 1), :, 2 * jg + jl],
                            psq[64 * jl:64 * (jl + 1),
                                256 * b:256 * (b + 1)])

            # ---- v projection -> vstage -> vh gather --------------------
            # vh_all[b] [128, 16*65]: tile t at cols 65t:65t+64, ones at 65t+64
            vh_all = []
            for b in range(B):
                t = vhp.tile([128, NJT * (DK + 1)], bf16, tag=f"vha{b}")
                vv = t.rearrange("p (t c) -> p t c", c=DK + 1)
                nc.gpsimd.memset(vv[:, :, DK:DK + 1], 1.0)
                vh_all.append(t)
            vh_v = [t.rearrange("p (t c) -> p t c", c=DK + 1) for t in vh_all]
            for rh in range(4):           # rh = 2*b + half
                psv = psM.tile([128, D], f32, tag=f"psf{rh % 2}")
                b, half = rh // 2, rh % 2
                for k in range(KT):
                    nc.tensor.matmul(
                        psv[:],
                        vT_sb[k][:, 256 * b + 128 * half:
                                 256 * b + 128 * (half + 1)],
                        wv_sb[k][:], start=(k == 0), stop=(k == KT - 1))
                vc = qcp.tile([128, D], bf16, tag="vc")
                nc.vector.tensor_copy(vc[:], psv[:])
                nc.sync.dma_start(vstage[rh, :, :], vc[:])
                # gather 8 vh tiles: dst[p=(8*rm+c8), d]
                #   <- vstage[rh][16*tl+rm, 64*c8+d]
                # (dst flat order (rm, c8, d) matches src (rm, (c8 d)))
                tb = 8 * half
                src = vstage[rh].rearrange("(tl rm) f -> tl rm f", tl=8)
                for tl in range(8):
                    nc.sync.dma_start(vh_v[b][:, tb + tl, 0:DK], src[tl])

            # ---- attention over i-chunks --------------------------------
            for n in range(NCHUNK):
                n_t = 4 * n + 4
                psx0 = psX.tile([DK + 1, CHUNK], f32, tag="psx0")
                psx1 = psX.tile([DK + 1, CHUNK], f32, tag="psx1")
                for t_ in range(n_t):
                    s_ = t_ - 4 * n
                    off = max(0, s_) * JT
                    pss = psS.tile([128, 2 * CHUNK], f32, tag="pss")
                    nc.tensor.matmul(
                        pss[:, off:CHUNK],
                        qhT[0:64, JT * t_:JT * (t_ + 1)],
                        qhT[0:64, CHUNK * n + off:CHUNK * (n + 1)],
                        start=True, stop=True, tile_position=(0, 0))
                    nc.tensor.matmul(
                        pss[:, CHUNK + off:2 * CHUNK],
                        qhT[64:128, JT * t_:JT * (t_ + 1)],
                        qhT[64:128, CHUNK * n + off:CHUNK * (n + 1)],
                        start=True, stop=True, tile_position=(64, 0))
                    eT = eTp.tile([128, 2 * CHUNK], bf16, tag="eT")
                    nc.scalar.activation(
                        eT[:, off:], pss[:, off:],
                        mybir.ActivationFunctionType.Exp,
                        bias=exp_bias[:], scale=1.0 / math.sqrt(DK))
                    if s_ >= 0:
                        # only the diagonal 128-col block needs masking
                        sl0 = eT[:, off:off + JT]
                        nc.vector.tensor_mul(sl0, sl0, mt_sb[:])
                        sl1 = eT[:, CHUNK + off:CHUNK + off + JT]
                        nc.gpsimd.tensor_mul(sl1, sl1, mt_sb[:])
                    nc.tensor.matmul(
                        psx0[:, off:], vh_all[0][:, 65 * t_:65 * t_ + 65],
                        eT[:, off:CHUNK],
                        start=(t_ == 0), stop=(t_ == n_t - 1),
                        skip_group_check=True)
                    nc.tensor.matmul(
                        psx1[:, off:], vh_all[1][:, 65 * t_:65 * t_ + 65],
                        eT[:, CHUNK + off:2 * CHUNK],
                        start=(t_ == 0), stop=(t_ == n_t - 1),
                        skip_group_check=True)

                # ---- chunk epilogue: evacuate x^T + l, project, store ---
                xT = xTp.tile([128, CHUNK], bf16, tag="xT")
                nc.vector.tensor_copy(xT[0:64, :], psx0[0:64, :])
                nc.vector.tensor_copy(xT[64:128, :], psx1[0:64, :])
                ls0 = xTp.tile([1, CHUNK], f32, tag="ls0")
                ls1 = xTp.tile([1, CHUNK], f32, tag="ls1")
                nc.vector.tensor_copy(ls0[:], psx0[64:65, :])
                nc.vector.tensor_copy(ls1[:], psx1[64:65, :])
                nc.sync.dma_start(lo[0, :, CHUNK * n:CHUNK * (n + 1)],
                                  ls0[:])
                nc.sync.dma_start(lo[1, :, CHUNK * n:CHUNK * (n + 1)],
                                  ls1[:])
                for u in range(CHUNK // 128):
                    psf0 = psM.tile([128, D], f32, tag="psf0")
                    psf1 = psM.tile([128, D], f32, tag="psf1")
                    nc.tensor.matmul(
                        psf0[:], xT[0:64, 128 * u:128 * (u + 1)],
                        wo_sb[0:64, :], start=True, stop=True,
                        tile_position=(0, 0))
                    nc.tensor.matmul(
                        psf1[:], xT[64:128, 128 * u:128 * (u + 1)],
                        wo_sb[64:128, :], start=True, stop=True,
                        tile_position=(64, 0))
                    fo0 = fop.tile([128, D], bf16, tag="fo0")
                    fo1 = fop.tile([128, D], bf16, tag="fo1")
                    nc.vector.tensor_copy(fo0[:], psf0[:])
                    nc.vector.tensor_copy(fo1[:], psf1[:])
                    r0 = CHUNK * n + 128 * u
                    nc.sync.dma_start(po[0, r0:r0 + 128, :], fo0[:])
                    nc.sync.dma_start(po[1, r0:r0 + 128, :], fo1[:])
    nc.compile()
    return nc


def _tri_pattern():
    import ml_dtypes
    p = np.arange(JT)[:, None]
    f = np.arange(JT)[None, :]
    return (p <= f).astype(ml_dtypes.bfloat16)


def _numpy_fallback(query, key, value, mask, Wq, Wk, Wv, Wo):
    q = query @ Wq.T
    v = value @ Wv.T
    qh = q.reshape(B, H, S, DK)
    vh = v.reshape(B, H, S, DK)
    scores = np.einsum("bhqd,bhkd->bhqk", qh, qh) / math.sqrt(DK)
    scores = np.where(mask == 0, np.float32(-1e9), scores)
    scores = scores - scores.max(axis=-1, keepdims=True)
    e = np.exp(scores)
    attn = e / e.sum(axis=-1, keepdims=True)
    x = np.einsum("bhqk,bhkd->bhqd", attn, vh)
    x = x.transpose(0, 2, 1, 3).reshape(B, S, H * DK)
    return (x @ Wo.T).astype(np.float32)


def _run_device(query, value, Wq, Wv, Wo, trace=False):
    import ml_dtypes
    from concourse.bass_utils import run_bass_kernel_spmd

    if "nc" not in _cache:
        _cache["nc"] = _build_causal()
    nc = _cache["nc"]

    bf = ml_dtypes.bfloat16
    mtri = _tri_pattern()
    wqT = np.ascontiguousarray(Wq.T).astype(bf)
    wvT = np.ascontiguousarray(Wv.T).astype(bf)
    in_maps = []
    for c in range(NCORES):
        r0 = SLAB * c
        qs = query[:, r0:r0 + SLAB, :]      # [B, SLAB, D]
        vs = value[:, r0:r0 + SLAB, :]
        in_maps.append({
            # [D, 2*SLAB]: b0 cols then b1 cols
            "qT": np.ascontiguousarray(
                qs.transpose(2, 0, 1).reshape(D, 2 * SLAB)).astype(bf),
            "vT": np.ascontiguousarray(
                vs.transpose(2, 0, 1).reshape(D, 2 * SLAB)).astype(bf),
            "wqT": wqT,
            "wvT": wvT,
            "woT": np.ascontiguousarray(
                Wo[:, DK * c:DK * (c + 1)].T).astype(bf),
            "mtri": mtri,
        })
    res = run_bass_kernel_spmd(nc, in_maps, core_ids=list(range(NCORES)),
                               trace=trace)
    out = np.zeros((B, S, D), dtype=np.float32)
    for c in range(NCORES):
        pc = res.results[c]
        out += pc["po"].astype(np.float32) / \
            pc["lo"].reshape(B, S, 1)
    return out, res


_TRIL = None


def kernel(query, key, value, mask, Wq, Wk, Wv, Wo):
    global _TRIL
    query = np.asarray(query, dtype=np.float32)
    value = np.asarray(value, dtype=np.float32)
    mask = np.asarray(mask)
    Wq = np.asarray(Wq, dtype=np.float32)
    Wv = np.asarray(Wv, dtype=np.float32)
    Wo = np.asarray(Wo, dtype=np.float32)

    if _TRIL is None:
        _TRIL = np.tril(np.ones((S, S), dtype=np.int64))
    m2 = mask.reshape(S, S)
    if not np.array_equal(m2 != 0, _TRIL != 0):
        return _numpy_fallback(query, np.asarray(key), value, mask,
                               Wq, np.asarray(Wk), Wv, Wo)

    out, _ = _run_device(query, value, Wq, Wv, Wo)
    return out
